# revision 51
# baseline (speedup 1.0000x reference)
"""Gated multi-head self-attention on 8 Trainium2 NeuronCores.

Reference computation (per batch b of 4, N=1024 tokens, 8 heads x 64):
    q  = (x @ wq.T) * 64**-0.5
    k,v = split(x @ wkv.T)
    dots = q k^T + bias;  attn = softmax(dots)
    out  = (attn @ v) * sigmoid(x @ wg.T + bg)
    y    = out @ wo.T + bo                # bo added on host after gather

Sharding: token-sharded, zero collectives. Core c handles batch b=c//2 and
query-token half c%2 (512 query rows). Each core computes K/V for its whole
batch.

Schedule notes (v2, ~70.5us vs the 78.0us f16-V baseline):
  - Q/K/G/V projections all run in fp8e4 DoubleRow mode; weights pre-scaled
    by 64 on the host. The Q/K descale is folded into the Exp activation's
    `scale`; the V descale is folded into the softmax-denominator ones
    columns (memset to 64 instead of 1). fp8 V costs rel_err 7.5e-3 ->
    1.40e-2 (gate is 2e-2) and saves ~3.4us of PE plus 1.5MB of DMA.
  - ALL input DMAs ride ONE HWDGE queue in consumption order: f8a1 (x8
    query-half + ct0 weights, 0.39MB -- everything dots(0,0..3) need),
    f8a2 (x8 kv-half), f8a3 (remaining K/Q weights), f8b (wg8|wv8), then
    the 16-load exp-bias ring (0.52MB each, dispatched 2 ahead of use).
    One queue avoids the SDMA per-packet round-robin, where a large-packet
    bulk transfer starves small-packet just-in-time loads (~86/14 split)
    and delays completion semaphores via straggler packets. Only bgn/woT
    use the SWDGE queue, gated behind f8b by a guard copy so Tile cannot
    hoist their dispatch into the startup window.
  - Warmup matmuls cover the f8a1 wait so HAM stays at K=8/8 end-to-end.
  - Softmax is unnormalized exp multiplied by exp(bias^T) (host fp16); the
    denominators come free from the 64-value ones-columns appended to V.
  - Phase 1 runs V before gates/eg so the DVE FIFO stays [casts | eb-mults]
    and the phase-2 ps_av WAR never waits behind a clogged vector queue;
    eg exps read the last proj-pool psum tiles on the scalar engine.
  - Phase 2 emits [dots, av, av] per slot (dots first: the exp stream must
    not queue behind av catch-up); eb-mults split vector/gpsimd by a
    static set sized to gpsimd's ~2.2us/tile tensor_tensor (its
    tensor_scalar path is ~7.5us/tile -- never use it on the hot path).
  - Tail: gating chain (2 cden, recip, 2 gated) is DVE-throughput-bound;
    O-projection ct0-2 and the last av steps overlap it; ct3 is split into
    64-row halves so its matmuls start as soon as gated3's s0 lands; y
    casts alternate scalar/vector per it-tile, each followed immediately
    by its own 0.25MB DMA on alternating HWDGE queues.
  - PSUM is a two-sided stack: dots(4 banks) on the right (its banks become
    the y accumulators), proj(4) -> av(4) sequential on the left.
"""

import sys

if "/opt/trn_rl_repo" not in sys.path:
    sys.path.insert(0, "/opt/trn_rl_repo")

import ml_dtypes
import numpy as np

import concourse.bass as bass  # noqa: F401  (AP helpers)
import concourse.mybir as mybir
import concourse.tile as tile
from concourse import bacc
from concourse.bass_utils import run_bass_kernel_spmd

F32 = mybir.dt.float32
F16 = mybir.dt.float16
F8 = mybir.dt.float8e4
AF = mybir.ActivationFunctionType
ALU = mybir.AluOpType
DR = mybir.MatmulPerfMode.DoubleRow
FP8_NP = ml_dtypes.float8_e4m3

P = 128
HEADS = 8
DH = 64
DIM = 512
N = 1024  # tokens per batch (kv length)
NQ = 512  # query tokens per core
B = 4
N_CORES = 8
DT = DIM // P  # 4 channel tiles
JT = N // P  # 8 kv-token tiles
HP = HEADS // 2  # 4 head pairs

W_SCALE = 64.0  # host-side fp8 weight scale for wq/wk/wg/wv
EXP_SCALE = 1.0 / (W_SCALE * W_SCALE * 8.0)  # descale + dim_head**-0.5

N_WARM = 64
# eb-mult tiles run on gpsimd for these (hp, jt) to keep the DVE under the
# exp cadence; hp3 stays on vector (gpsimd would stretch the tail)
GP_MULT = {(0, 3), (0, 5), (0, 6), (1, 3), (1, 5), (2, 2), (2, 3), (3, 2), (3, 3)}


def build_nc():
    nc = bacc.Bacc(None, target_bir_lowering=False, debug=False)

    # Per-core inputs. Token order inside x is "query half first".
    # f8a packs x8 | wk8 | wq8 column-wise (the critical startup load);
    # f8b packs wg8 | wv8 (first needed ~8us later)
    F8A = N + 2 * DIM
    F8B = 2 * DIM
    f8a_d = nc.dram_tensor("f8a", [DIM, F8A], F8, kind="ExternalInput")
    f8b_d = nc.dram_tensor("f8b", [DIM, F8B], F8, kind="ExternalInput")
    woT_d = nc.dram_tensor("woT", [DIM, DIM], F16, kind="ExternalInput")
    bgn_d = nc.dram_tensor("bgn", [DIM], F32, kind="ExternalInput")  # -bg
    bT_d = nc.dram_tensor("bT", [HP, N // 256, P, 2, 2 * NQ], F16, kind="ExternalInput")
    y_d = nc.dram_tensor("y", [NQ, DIM], F16, kind="ExternalOutput")

    with tile.TileContext(nc) as tc:
        with (
            tc.tile_pool(name="const", bufs=1) as const,
            tc.tile_pool(name="work", bufs=1) as work,
            tc.tile_pool(name="attn", bufs=20) as attn_pool,
            tc.tile_pool(name="rec", bufs=2) as rec_pool,
            tc.tile_pool(name="ebuf", bufs=8) as ebuf,
            tc.tile_pool(name="yout", bufs=4) as yout,
        ):
            # ---- constants; DMA queues by priority -----------------------
            warm_sb = const.tile([P, P], F16, tag="warm", name="warm")
            nc.vector.memset(warm_sb[:], 1.0)

            # sync (HWDGE) queue, in consumption order: f8a head (x8 + ct0
            # weights -- everything the first projections/dots need), f8a
            # tail (ct1-3 weights), f8b (gates/value weights), then the
            # just-in-time bias ring. Pack layout:
            #   f8a: x8 | wk8_c0 | wq8_c0 | wk8_c1 wk8_c2 wk8_c3 | wq8_c1..3
            # f8a split into three tiles by first-use time (separate tiles,
            # not sliced DMAs: tile-level dependency tracking would make
            # every consumer wait for all transfers):
            #   f8a1 = x8 query-half + ct0 weights  (everything dots(0,0..3)
            #          needs -- 0.39MB, lands ~1.5us before the full pack)
            #   f8a2 = x8 kv-half
            #   f8a3 = ct1-3 weights
            HEAD = NQ + 2 * P
            f8a_r = f8a_d.rearrange("(o p) m -> p o m", p=P)
            f8a1 = const.tile([P, DT, HEAD], F8, tag="f8a1", name="f8a1")
            nc.sync.dma_start(f8a1[:], f8a_r[:, :, 0:HEAD])
            f8a2 = const.tile([P, DT, NQ], F8, tag="f8a2", name="f8a2")
            nc.sync.dma_start(f8a2[:], f8a_r[:, :, HEAD : HEAD + NQ])
            f8a3 = const.tile([P, DT, 6 * P], F8, tag="f8a3", name="f8a3")
            nc.sync.dma_start(f8a3[:], f8a_r[:, :, HEAD + NQ : F8A])
            f8b = const.tile([P, DT, F8B], F8, tag="f8b", name="f8b")
            nc.sync.dma_start(f8b[:], f8b_d.rearrange("(o p) m -> p o m", p=P))

            def x8_ap(kp, lo, hi):
                # x token columns: [0:NQ) in f8a1, [NQ:N) in f8a2
                if hi <= NQ:
                    return f8a1[:, kp : kp + 2, lo:hi]
                return f8a2[:, kp : kp + 2, lo - NQ : hi - NQ]

            wk_ap = [f8a1[:, :, NQ : NQ + P]] + [
                f8a3[:, :, ct * P : (ct + 1) * P] for ct in range(3)
            ]
            wq_ap = [f8a1[:, :, NQ + P : NQ + 2 * P]] + [
                f8a3[:, :, (3 + ct) * P : (4 + ct) * P] for ct in range(3)
            ]
            wg8 = f8b[:, :, 0:DIM]
            wv8 = f8b[:, :, DIM : 2 * DIM]

            # gpsimd (SWDGE) queue: only the tiny gating bias now, plus woT
            # dispatched behind the f8b guard (emit_hi_dmas)
            bgn_sb = const.tile([P, DT], F32, tag="bgn", name="bgn")
            nc.gpsimd.dma_start(bgn_sb[:], bgn_d.rearrange("(o p) -> p o", p=P))

            woT = const.tile([P, DT, DIM], F16, tag="woT", name="woT")

            def emit_hi_dmas():
                # woT rides the (otherwise idle) SWDGE queue; the guard copy
                # READS f8b so Tile can't hoist the dispatch ahead of the
                # startup-critical loads (its packets would steal bandwidth
                # from f8a/f8b stragglers and delay their semaphores)
                nc.gpsimd.tensor_copy(
                    out=woT[0:1, :, 0:1], in_=f8b[0:1, 0, 0:4]
                )
                nc.gpsimd.dma_start(
                    woT[:], woT_d.rearrange("(o p) m -> p o m", p=P)
                )

            # persistent activations
            kT = [work.tile([P, N], F16, tag=f"kT{t}", name=f"kT{t}") for t in range(DT)]
            qT = [work.tile([P, NQ], F16, tag=f"qT{t}", name=f"qT{t}") for t in range(DT)]
            v_aug = [work.tile([P, HEADS * P], F16, tag=f"vaug{j}", name=f"vaug{j}") for j in range(JT)]
            egT = [work.tile([P, NQ], F16, tag=f"eg{t}", name=f"eg{t}") for t in range(DT)]
            gatedT = [work.tile([P, NQ], F16, tag=f"gated{t}", name=f"gated{t}") for t in range(DT)]

            # denominator columns (gpsimd: SBUF only). Value 64 (not 1)
            # absorbs the fp8 V weight scale: av_v and denom both carry the
            # 64x factor, which cancels in av_v / (denom * egp1).
            for jt in range(JT):
                nc.gpsimd.memset(
                    v_aug[jt].rearrange("p (h c) -> p h c", c=P)[:, :, DH:P], W_SCALE
                )

            # exp-bias ring: ALL bias tiles stream just-in-time on the sync
            # HWDGE queue in consumption order, two loads ahead of use.
            # One queue, one ordering: no SWDGE bulk transfer whose large
            # packets would beat the ring in the SDMA per-packet round-robin
            eb_ring = {}
            eb_next = {"k": 0}
            tiles_by_hp = {}

            def dispatch_eb():
                k = eb_next["k"]
                if k < 2 * JT:
                    t = ebuf.tile([P, 2, 2 * NQ], F16, tag="eb", name="eb")
                    nc.sync.dma_start(t[:], bT_d[k // 4, k % 4])
                    eb_ring[k] = t
                    eb_next["k"] = k + 1

            def emit_dots_tile(hp, jt):
                ct = hp
                if jt % 2 == 0:
                    dispatch_eb()
                eb = eb_ring[hp * 4 + jt // 2][:, jt % 2, :]
                dps = ps_dots.tile([P, 2 * NQ], F32, tag="dots", name="dots")
                for s in range(2):
                    lo = s * DH
                    nc.tensor.matmul(
                        dps[:, s * NQ : (s + 1) * NQ],
                        kT[ct][lo : lo + DH, jt * P : (jt + 1) * P],
                        qT[ct][lo : lo + DH, :],
                        start=True,
                        stop=True,
                        tile_position=(lo, 0),
                    )
                at = attn_pool.tile([P, 2 * NQ], F16, tag="attn", name="attn")
                nc.scalar.activation(out=at[:], in_=dps[:], func=AF.Exp, scale=EXP_SCALE)
                meng = nc.gpsimd if (hp, jt) in GP_MULT else nc.vector
                meng.tensor_tensor(at[:], at[:], eb[:], ALU.mult)
                tiles_by_hp.setdefault(hp, []).append(at)
                return at

            def emit_kq_proj(w_ap, dst, jc, nq, cast_eng):
                ps = ps_proj.tile([P, NQ], F32, tag="proj", name="proj")
                for kp in (0, 2):
                    nc.tensor.matmul(
                        ps[:],
                        w_ap[:, kp : kp + 2, :],
                        x8_ap(kp, jc * NQ, jc * NQ + nq),
                        start=(kp == 0),
                        stop=(kp == 2),
                        perf_mode=DR,
                    )
                if cast_eng is nc.scalar:
                    nc.scalar.activation(
                        out=dst[:, jc * NQ : (jc + 1) * NQ], in_=ps[:], func=AF.Copy
                    )
                else:
                    cast_eng.tensor_copy(out=dst[:, jc * NQ : (jc + 1) * NQ], in_=ps[:])

            def emit_gates(ct):
                ps = ps_proj.tile([P, NQ], F32, tag="proj", name="proj")
                for kp in (0, 2):
                    nc.tensor.matmul(
                        ps[:],
                        wg8[:, kp : kp + 2, ct * P : (ct + 1) * P],
                        x8_ap(kp, 0, NQ),
                        start=(kp == 0),
                        stop=(kp == 2),
                        perf_mode=DR,
                    )
                return ps  # scalar exp emitted separately (emit_eg)

            def emit_eg(ct, ps):
                # eg = exp(-(g + bg)); gates are fp8 with weights scaled by
                # 64, so the exp descale is -1/64. Reuses the Exp table.
                nc.scalar.activation(
                    out=egT[ct][:], in_=ps[:], func=AF.Exp, scale=-1.0 / W_SCALE,
                    bias=bgn_sb[:, ct : ct + 1],
                )
                # egp1 = 1 + eg, in place (f16, 2x DVE mode). NOT gpsimd:
                # its tensor_scalar software path measures ~7.5us/tile.
                nc.vector.tensor_scalar_add(egT[ct][:], egT[ct][:], 1.0)

            def emit_v(jt):
                # fp8 DoubleRow: v_psum = 64 * v  (descale via ones=64)
                ps = ps_proj.tile([P, NQ], F32, tag="proj", name="proj")
                for kp in (0, 2):
                    nc.tensor.matmul(
                        ps[:],
                        x8_ap(kp, jt * P, (jt + 1) * P),
                        wv8[:, kp : kp + 2, :],
                        start=(kp == 0),
                        stop=(kp == 2),
                        perf_mode=DR,
                    )
                return ps

            def emit_v_cast(jt, ps):
                nc.vector.tensor_copy(
                    out=v_aug[jt].rearrange("p (h c) -> p h c", c=P)[:, :, 0:DH],
                    in_=ps[:].rearrange("p (h c) -> p h c", c=DH),
                )

            def emit_av_jt(av, hp, jt):
                tiles = tiles_by_hp[hp]
                for s in range(2):
                    h = 2 * hp + s
                    nc.tensor.matmul(
                        av[:, s * NQ : (s + 1) * NQ],
                        v_aug[jt][:, h * P : (h + 1) * P],
                        tiles[jt][:, s * NQ : (s + 1) * NQ],
                        start=(jt == 0),
                        stop=(jt == JT - 1),
                    )

            # gated = av[v] / (denom * (1 + e^-g)) ; one fast reciprocal.
            # Split into three pieces so hp2's chain can be spread across
            # the last phase-2 slots (5 back-to-back DVE ops would delay
            # the hp3 eb-mults behind them in the DVE FIFO, stalling the
            # final exps)
            def emit_gating_a(av, hp):
                cden = rec_pool.tile([P, NQ], F32, tag="cden", name="cden")
                for s in range(2):
                    lo = s * DH
                    nc.vector.tensor_tensor(
                        cden[lo : lo + DH, :],
                        av[DH:P, s * NQ : (s + 1) * NQ],
                        egT[hp][lo : lo + DH, :],
                        ALU.mult,
                    )
                return cden

            def emit_gating_b(cden):
                crec = rec_pool.tile([P, NQ], F32, tag="crec", name="crec")
                nc.vector.reciprocal_approx_fast(out=crec[:], in_=cden[:])
                return crec

            def emit_gating_c(av, hp, crec):
                for s in range(2):
                    lo = s * DH
                    nc.vector.tensor_tensor(
                        gatedT[hp][lo : lo + DH, :],
                        av[0:DH, s * NQ : (s + 1) * NQ],
                        crec[lo : lo + DH, :],
                        ALU.mult,
                    )

            def emit_gating(av, hp):
                emit_gating_c(av, hp, emit_gating_b(emit_gating_a(av, hp)))

            # av-step iterator state: one AV accumulation step = both heads
            # of one kv tile; gating is emitted right after a pair completes
            av_state = {"a": 0, "tiles": {}}

            def emit_av_step(ps_av):
                a = av_state["a"]
                hp, jt = divmod(a, JT)
                if jt == 0:
                    av_state["tiles"][hp] = ps_av.tile(
                        [P, 2 * NQ], F32, tag="av", name="av"
                    )
                av = av_state["tiles"][hp]
                emit_av_jt(av, hp, jt)
                if jt == JT - 1:
                    emit_gating(av, hp)
                av_state["a"] = a + 1

            # PSUM is a two-sided stack: dots lives on the right and closes
            # mid-stream (its banks become the y accumulators); proj -> av
            # run sequentially on the left. Manual pool lifetimes keep both
            # sides at <= 4 banks, 8 total.
            ctx_dots = tc.tile_pool(name="ps_dots", bufs=2, space="PSUM", side="right")
            ps_dots = ctx_dots.__enter__()
            if True:
                if True:
                    # ---- phase 1: projections + dots(hp0, hp1) -----------
                    with tc.tile_pool(
                        name="ps_proj", bufs=4, space="PSUM", side="left"
                    ) as ps_proj:
                        # dummy 1x1 exp: pulls the scalar engine's
                        # ACT_TABLE_LOAD into the DMA wait at startup
                        nc.scalar.activation(
                            out=egT[0][0:1, 0:1], in_=warm_sb[0:1, 0:1],
                            func=AF.Exp,
                        )
                        warm_ps = ps_proj.tile([P, NQ], F32, tag="proj", name="proj")
                        for _ in range(N_WARM):
                            nc.tensor.matmul(
                                warm_ps[:, 0:P], warm_sb[:], warm_sb[:],
                                start=True, stop=True,
                            )
                        nc.scalar.activation(
                            out=warm_sb[0:1, 0:1], in_=warm_ps[0:1, 0:1],
                            func=AF.Copy,
                        )

                        # prime the bias ring (2 loads in flight ahead of use)
                        dispatch_eb()
                        dispatch_eb()

                        # qT0 cast on scalar (no exps exist yet); k casts on
                        # vector. All hp0 dots only need ct0; the ps_dots
                        # WAR ring paces the PE to the exp stream and
                        # fillers slot into the slack.
                        emit_kq_proj(wq_ap[0], qT[0], 0, NQ, nc.scalar)
                        emit_kq_proj(wk_ap[0], kT[0], 0, NQ, nc.vector)
                        emit_kq_proj(wk_ap[0], kT[0], 1, NQ, nc.vector)
                        emit_dots_tile(0, 0)
                        emit_dots_tile(0, 1)
                        emit_dots_tile(0, 2)
                        for ct in range(1, DT):
                            emit_kq_proj(wk_ap[ct], kT[ct], 0, NQ, nc.vector)
                            emit_kq_proj(wk_ap[ct], kT[ct], 1, NQ, nc.vector)
                            emit_kq_proj(wq_ap[ct], qT[ct], 0, NQ, nc.vector)
                            emit_dots_tile(0, 2 + ct)
                        # V-block first (its casts keep the DVE FIFO light),
                        # gates/eg at the end: the eg exps read the last
                        # proj-pool tiles on the SCALAR engine, so the
                        # phase-2 ps_av WAR never waits on a clogged DVE
                        emit_v_cast(0, emit_v(0))
                        emit_dots_tile(0, 6)
                        emit_v_cast(1, emit_v(1))
                        emit_dots_tile(0, 7)
                        emit_v_cast(2, emit_v(2))
                        emit_dots_tile(1, 0)
                        emit_v_cast(3, emit_v(3))
                        emit_dots_tile(1, 1)
                        emit_v_cast(4, emit_v(4))
                        emit_dots_tile(1, 2)
                        emit_v_cast(5, emit_v(5))
                        emit_hi_dmas()
                        emit_dots_tile(1, 3)
                        emit_v_cast(6, emit_v(6))
                        emit_dots_tile(1, 4)
                        emit_v_cast(7, emit_v(7))
                        emit_dots_tile(1, 5)
                        emit_eg(0, emit_gates(0))
                        emit_dots_tile(1, 6)
                        emit_eg(1, emit_gates(1))
                        emit_dots_tile(1, 7)
                        emit_eg(2, emit_gates(2))
                        emit_eg(3, emit_gates(3))

                    # ---- phase 2: zip remaining dots with AV steps -------
                    # [dots, av, av]: the AV stream (32 steps, starting 16
                    # behind) catches up to the exps by the end, so the tail
                    # after the last exp is just the final gating chain.
                    ctx_av = tc.tile_pool(
                        name="ps_av", bufs=2, space="PSUM", side="left"
                    )
                    ps_av = ctx_av.__enter__()
                    # dots FIRST in each slot: the exp stream must never
                    # queue behind the av catch-up in the PE FIFO
                    # dots tiles go out in PAIRS (the 2-deep psum ring allows
                    # two outstanding): dots(n+1) is no longer queued behind
                    # av steps in the PE FIFO, so the exp stream isn't
                    # stretched by the +0.86us av batch per ring handoff
                    d_emitted = 16
                    for hp in (2, 3):
                        for jt in range(JT):
                            emit_dots_tile(hp, jt)
                            d_emitted += 1
                            for _ in range(2):
                                if av_state["a"] <= d_emitted - 2:
                                    emit_av_step(ps_av)

            # dots pool closes here; its right-side banks become the y tiles
            ctx_dots.__exit__(None, None, None)
            ctx_y = tc.tile_pool(name="ps_y", bufs=1, space="PSUM", side="right")
            ps_y = ctx_y.__enter__()

            # ---- phase 3: output projection tail -------------------------
            # ct0..2 interleave with the remaining AV steps: the PE fills
            # its exp-wait slack with output-projection work
            ys = [ps_y.tile([P, DIM], F32, tag=f"y{it}", name="y") for it in range(4)]
            for ct in range(DT - 1):
                for it in range(NQ // P):
                    nc.tensor.matmul(
                        ys[it][:],
                        gatedT[ct][:, it * P : (it + 1) * P],
                        woT[:, ct, :],
                        start=(ct == 0),
                        stop=False,
                    )
                for _ in range(2):
                    if av_state["a"] < HP * JT:
                        emit_av_step(ps_av)
            # ct3 split into two 64-row halves (tile_position pairs) so the
            # it-tiles can start as soon as gated3's s0 half lands; y casts
            # pipeline with the s1 matmuls
            ysb = yout.tile([P, NQ // P, DIM], F16, tag="ysb", name="ysb")
            ydst = y_d.rearrange("(f p) m -> p f m", p=P)
            for s in range(2):
                lo = s * DH
                for it in range(NQ // P):
                    nc.tensor.matmul(
                        ys[it][:],
                        gatedT[DT - 1][lo : lo + DH, it * P : (it + 1) * P],
                        woT[lo : lo + DH, DT - 1, :],
                        start=False,
                        stop=(s == 1),
                        tile_position=(lo, 0),
                    )
                    if s == 1:
                        if it % 2 == 0:
                            nc.scalar.activation(
                                out=ysb[:, it, :], in_=ys[it][:], func=AF.Copy
                            )
                            nc.scalar.dma_start(
                                ydst[:, it : it + 1, :], ysb[:, it : it + 1, :]
                            )
                        else:
                            nc.vector.tensor_copy(out=ysb[:, it, :], in_=ys[it][:])
                            nc.sync.dma_start(
                                ydst[:, it : it + 1, :], ysb[:, it : it + 1, :]
                            )
            ctx_y.__exit__(None, None, None)
            ctx_av.__exit__(None, None, None)

    nc.compile()
    return nc


_CACHE = {}


def get_nc():
    if "nc" not in _CACHE:
        _CACHE["nc"] = build_nc()
    return _CACHE["nc"]


def make_in_maps(x, attn_bias, wq, wkv, wo, wg, bg):
    """Host-side sharding: per-core input dicts (weights shared by reference)."""
    x = np.asarray(x, np.float32)
    attn_bias = np.asarray(attn_bias, np.float32)
    wqT = np.asarray(wq, np.float32).T
    wkvT = np.asarray(wkv, np.float32).T
    wq8 = (wqT * W_SCALE).astype(FP8_NP)
    wk8 = (wkvT[:, :DIM] * W_SCALE).astype(FP8_NP)
    wv8 = (wkvT[:, DIM:] * W_SCALE).astype(FP8_NP)
    wg8 = (np.asarray(wg, np.float32).T * W_SCALE).astype(FP8_NP)
    woT = np.ascontiguousarray(np.asarray(wo, np.float32).T, np.float16)
    bgn = -np.asarray(bg, np.float32)
    f8b = np.ascontiguousarray(np.concatenate([wg8, wv8], axis=1))

    ab = np.exp(attn_bias[0])  # [H, N(i), N(j)]
    # bT[r0][h, j, i] = exp(bias)[h, i, j] with j permuted "query half first"
    bT = {}
    for r0 in (0, NQ):
        perm = np.r_[r0 : r0 + NQ, (NQ - r0) : (NQ - r0) + NQ]
        t = ab[:, r0 : r0 + NQ, :].transpose(0, 2, 1)[:, perm, :]
        t = t.reshape(4, 2, 4, 2, 128, NQ).transpose(0, 2, 4, 3, 1, 5)
        # t: [hp, pr, P, jt%2, s, NQ]
        t = np.ascontiguousarray(t, dtype=np.float16).reshape(4, 4, P, 2, 2 * NQ)
        bT[r0] = t

    in_maps = []
    for c in range(N_CORES):
        b, r0 = c // 2, (c % 2) * NQ
        perm = np.r_[r0 : r0 + NQ, (NQ - r0) : (NQ - r0) + NQ]
        xperm = x[b][perm].T
        x8 = xperm.astype(FP8_NP)
        f8a = np.concatenate(
            [
                x8[:, 0:NQ], wk8[:, 0:P], wq8[:, 0:P],  # head: lands first
                x8[:, NQ:],                              # x kv-half
                wk8[:, P:], wq8[:, P:],                  # ct1-3 weights
            ],
            axis=1,
        )
        in_maps.append(
            {
                "f8a": np.ascontiguousarray(f8a),
                "f8b": f8b,
                "woT": woT,
                "bgn": bgn,
                "bT": bT[r0],
            }
        )
    return in_maps


def kernel(x, mask, attn_bias, wq, wkv, wo, bo, wg, bg, **_):
    # mask is all-ones per the problem spec; ignored.
    nc = get_nc()
    in_maps = make_in_maps(x, attn_bias, wq, wkv, wo, wg, bg)
    res = run_bass_kernel_spmd(nc, in_maps, list(range(N_CORES))).results
    y = np.empty((B, N, DIM), np.float32)
    for c in range(N_CORES):
        b, r0 = c // 2, (c % 2) * NQ
        y[b, r0 : r0 + NQ] = res[c]["y"].astype(np.float32)
    y += np.asarray(bo, np.float32)
    return y


# revision 52
# speedup vs baseline: 1.0217x; 1.0217x over previous
"""Gated multi-head self-attention on 8 Trainium2 NeuronCores.

Reference computation (per batch b of 4, N=1024 tokens, 8 heads x 64):
    q  = (x @ wq.T) * 64**-0.5
    k,v = split(x @ wkv.T)
    dots = q k^T + bias;  attn = softmax(dots)
    out  = (attn @ v) * sigmoid(x @ wg.T + bg)
    y    = out @ wo.T + bo                # bo added on host after gather

Sharding: token-sharded, zero collectives. Core c handles batch b=c//2 and
query-token half c%2 (512 query rows). Each core computes K/V for its whole
batch.

Schedule notes (v2, ~70.5us vs the 78.0us f16-V baseline):
  - Q/K/G/V projections all run in fp8e4 DoubleRow mode; weights pre-scaled
    by 64 on the host. The Q/K descale is folded into the Exp activation's
    `scale`; the V descale is folded into the softmax-denominator ones
    columns (memset to 64 instead of 1). fp8 V costs rel_err 7.5e-3 ->
    1.40e-2 (gate is 2e-2) and saves ~3.4us of PE plus 1.5MB of DMA.
  - ALL input DMAs ride ONE HWDGE queue in consumption order: f8a1 (x8
    query-half + ct0 weights, 0.39MB -- everything dots(0,0..3) need),
    f8a2 (x8 kv-half), f8a3 (remaining K/Q weights), f8b (wg8|wv8), then
    the 16-load exp-bias ring (0.52MB each, dispatched 2 ahead of use).
    One queue avoids the SDMA per-packet round-robin, where a large-packet
    bulk transfer starves small-packet just-in-time loads (~86/14 split)
    and delays completion semaphores via straggler packets. Only bgn/woT
    use the SWDGE queue, gated behind f8b by a guard copy so Tile cannot
    hoist their dispatch into the startup window.
  - Warmup matmuls cover the f8a1 wait so HAM stays at K=8/8 end-to-end.
  - Softmax is unnormalized exp multiplied by exp(bias^T) (host fp16); the
    denominators come free from the 64-value ones-columns appended to V.
  - Phase 1 runs V before gates/eg so the DVE FIFO stays [casts | eb-mults]
    and the phase-2 ps_av WAR never waits behind a clogged vector queue;
    eg exps read the last proj-pool psum tiles on the scalar engine.
  - Phase 2 emits [dots, av, av] per slot (dots first: the exp stream must
    not queue behind av catch-up); eb-mults split vector/gpsimd by a
    static set sized to gpsimd's ~2.2us/tile tensor_tensor (its
    tensor_scalar path is ~7.5us/tile -- never use it on the hot path).
  - Tail: gating chain (2 cden, recip, 2 gated) is DVE-throughput-bound;
    O-projection ct0-2 and the last av steps overlap it; ct3 is split into
    64-row halves so its matmuls start as soon as gated3's s0 lands; y
    casts alternate scalar/vector per it-tile, each followed immediately
    by its own 0.25MB DMA on alternating HWDGE queues.
  - PSUM is a two-sided stack: dots(4 banks) on the right (its banks become
    the y accumulators), proj(4) -> av(4) sequential on the left.
"""

import sys

if "/opt/trn_rl_repo" not in sys.path:
    sys.path.insert(0, "/opt/trn_rl_repo")

import ml_dtypes
import numpy as np

import concourse.bass as bass  # noqa: F401  (AP helpers)
import concourse.mybir as mybir
import concourse.tile as tile
from concourse import bacc
from concourse.bass_utils import run_bass_kernel_spmd

F32 = mybir.dt.float32
F16 = mybir.dt.float16
F8 = mybir.dt.float8e4
AF = mybir.ActivationFunctionType
ALU = mybir.AluOpType
DR = mybir.MatmulPerfMode.DoubleRow
FP8_NP = ml_dtypes.float8_e4m3

P = 128
HEADS = 8
DH = 64
DIM = 512
N = 1024  # tokens per batch (kv length)
NQ = 512  # query tokens per core
B = 4
N_CORES = 8
DT = DIM // P  # 4 channel tiles
JT = N // P  # 8 kv-token tiles
HP = HEADS // 2  # 4 head pairs

W_SCALE = 64.0  # host-side fp8 weight scale for wq/wk/wg/wv
EXP_SCALE = 1.0 / (W_SCALE * W_SCALE * 8.0)  # descale + dim_head**-0.5

N_WARM = 64
# eb-mult tiles run on gpsimd for these (hp, jt) to keep the DVE under the
# exp cadence; hp3 stays on vector (gpsimd would stretch the tail)
GP_MULT = {(0, 3), (0, 5), (0, 6), (1, 3), (1, 5), (2, 2), (2, 3), (2, 5), (2, 6)}


def build_nc():
    nc = bacc.Bacc(None, target_bir_lowering=False, debug=False)

    # Per-core inputs. Token order inside x is "query half first".
    # f8a packs x8 | wk8 | wq8 column-wise (the critical startup load);
    # f8b packs wg8 | wv8 (first needed ~8us later)
    F8A = N + 2 * DIM
    F8B = 2 * DIM
    f8a_d = nc.dram_tensor("f8a", [DIM, F8A], F8, kind="ExternalInput")
    f8b_d = nc.dram_tensor("f8b", [DIM, F8B], F8, kind="ExternalInput")
    woT_d = nc.dram_tensor("woT", [DIM, DIM], F16, kind="ExternalInput")
    bgn_d = nc.dram_tensor("bgn", [DIM], F32, kind="ExternalInput")  # -bg
    bT_d = nc.dram_tensor("bT", [HP, N // 256, P, 2, 2 * NQ], F16, kind="ExternalInput")
    y_d = nc.dram_tensor("y", [NQ, DIM], F16, kind="ExternalOutput")

    with tile.TileContext(nc) as tc:
        with (
            tc.tile_pool(name="const", bufs=1) as const,
            tc.tile_pool(name="work", bufs=1) as work,
            tc.tile_pool(name="attn", bufs=20) as attn_pool,
            tc.tile_pool(name="rec", bufs=2) as rec_pool,
            tc.tile_pool(name="ebuf", bufs=8) as ebuf,
            tc.tile_pool(name="yout", bufs=4) as yout,
        ):
            # ---- constants; DMA queues by priority -----------------------
            warm_sb = const.tile([P, P], F16, tag="warm", name="warm")
            nc.vector.memset(warm_sb[:], 1.0)

            # sync (HWDGE) queue, in consumption order: f8a head (x8 + ct0
            # weights -- everything the first projections/dots need), f8a
            # tail (ct1-3 weights), f8b (gates/value weights), then the
            # just-in-time bias ring. Pack layout:
            #   f8a: x8 | wk8_c0 | wq8_c0 | wk8_c1 wk8_c2 wk8_c3 | wq8_c1..3
            # f8a split into three tiles by first-use time (separate tiles,
            # not sliced DMAs: tile-level dependency tracking would make
            # every consumer wait for all transfers):
            #   f8a1 = x8 query-half + ct0 weights  (everything dots(0,0..3)
            #          needs -- 0.39MB, lands ~1.5us before the full pack)
            #   f8a2 = x8 kv-half
            #   f8a3 = ct1-3 weights
            HEAD = NQ + 2 * P
            f8a_r = f8a_d.rearrange("(o p) m -> p o m", p=P)
            f8a1 = const.tile([P, DT, HEAD], F8, tag="f8a1", name="f8a1")
            nc.sync.dma_start(f8a1[:], f8a_r[:, :, 0:HEAD])
            f8a2 = const.tile([P, DT, NQ], F8, tag="f8a2", name="f8a2")
            nc.sync.dma_start(f8a2[:], f8a_r[:, :, HEAD : HEAD + NQ])
            f8a3 = const.tile([P, DT, 6 * P], F8, tag="f8a3", name="f8a3")
            nc.sync.dma_start(f8a3[:], f8a_r[:, :, HEAD + NQ : F8A])
            f8b = const.tile([P, DT, F8B], F8, tag="f8b", name="f8b")
            nc.sync.dma_start(f8b[:], f8b_d.rearrange("(o p) m -> p o m", p=P))

            def x8_ap(kp, lo, hi):
                # x token columns: [0:NQ) in f8a1, [NQ:N) in f8a2
                if hi <= NQ:
                    return f8a1[:, kp : kp + 2, lo:hi]
                return f8a2[:, kp : kp + 2, lo - NQ : hi - NQ]

            wk_ap = [f8a1[:, :, NQ : NQ + P]] + [
                f8a3[:, :, ct * P : (ct + 1) * P] for ct in range(3)
            ]
            wq_ap = [f8a1[:, :, NQ + P : NQ + 2 * P]] + [
                f8a3[:, :, (3 + ct) * P : (4 + ct) * P] for ct in range(3)
            ]
            wg8 = f8b[:, :, 0:DIM]
            wv8 = f8b[:, :, DIM : 2 * DIM]

            # gpsimd (SWDGE) queue: only the tiny gating bias now, plus woT
            # dispatched behind the f8b guard (emit_hi_dmas)
            bgn_sb = const.tile([P, DT], F32, tag="bgn", name="bgn")
            nc.gpsimd.dma_start(bgn_sb[:], bgn_d.rearrange("(o p) -> p o", p=P))

            woT = const.tile([P, DT, DIM], F16, tag="woT", name="woT")

            def emit_hi_dmas():
                # woT rides the (otherwise idle) SWDGE queue; the guard copy
                # READS f8b so Tile can't hoist the dispatch ahead of the
                # startup-critical loads (its packets would steal bandwidth
                # from f8a/f8b stragglers and delay their semaphores)
                nc.gpsimd.tensor_copy(
                    out=woT[0:1, :, 0:1], in_=f8b[0:1, 0, 0:4]
                )
                nc.gpsimd.dma_start(
                    woT[:], woT_d.rearrange("(o p) m -> p o m", p=P)
                )

            # persistent activations
            kT = [work.tile([P, N], F16, tag=f"kT{t}", name=f"kT{t}") for t in range(DT)]
            qT = [work.tile([P, NQ], F16, tag=f"qT{t}", name=f"qT{t}") for t in range(DT)]
            v_aug = [work.tile([P, HEADS * P], F16, tag=f"vaug{j}", name=f"vaug{j}") for j in range(JT)]
            egT = [work.tile([P, NQ], F16, tag=f"eg{t}", name=f"eg{t}") for t in range(DT)]
            gatedT = [work.tile([P, NQ], F16, tag=f"gated{t}", name=f"gated{t}") for t in range(DT)]

            # denominator columns (gpsimd: SBUF only). Value 64 (not 1)
            # absorbs the fp8 V weight scale: av_v and denom both carry the
            # 64x factor, which cancels in av_v / (denom * egp1).
            for jt in range(JT):
                nc.gpsimd.memset(
                    v_aug[jt].rearrange("p (h c) -> p h c", c=P)[:, :, DH:P], W_SCALE
                )

            # exp-bias ring: ALL bias tiles stream just-in-time on the sync
            # HWDGE queue in consumption order, two loads ahead of use.
            # One queue, one ordering: no SWDGE bulk transfer whose large
            # packets would beat the ring in the SDMA per-packet round-robin
            eb_ring = {}
            eb_next = {"k": 0}
            tiles_by_hp = {}

            def dispatch_eb():
                k = eb_next["k"]
                if k < 2 * JT:
                    t = ebuf.tile([P, 2, 2 * NQ], F16, tag="eb", name="eb")
                    nc.sync.dma_start(t[:], bT_d[k // 4, k % 4])
                    eb_ring[k] = t
                    eb_next["k"] = k + 1

            def emit_dots_tile(hp, jt):
                ct = hp
                if jt % 2 == 0:
                    dispatch_eb()
                eb = eb_ring[hp * 4 + jt // 2][:, jt % 2, :]
                dps = ps_dots.tile([P, 2 * NQ], F32, tag="dots", name="dots")
                for s in range(2):
                    lo = s * DH
                    nc.tensor.matmul(
                        dps[:, s * NQ : (s + 1) * NQ],
                        kT[ct][lo : lo + DH, jt * P : (jt + 1) * P],
                        qT[ct][lo : lo + DH, :],
                        start=True,
                        stop=True,
                        tile_position=(lo, 0),
                    )
                at = attn_pool.tile([P, 2 * NQ], F16, tag="attn", name="attn")
                nc.scalar.activation(out=at[:], in_=dps[:], func=AF.Exp, scale=EXP_SCALE)
                meng = nc.gpsimd if (hp, jt) in GP_MULT else nc.vector
                meng.tensor_tensor(at[:], at[:], eb[:], ALU.mult)
                tiles_by_hp.setdefault(hp, []).append(at)
                return at

            def emit_kq_proj(w_ap, dst, jc, nq, cast_eng):
                ps = ps_proj.tile([P, NQ], F32, tag="proj", name="proj")
                for kp in (0, 2):
                    nc.tensor.matmul(
                        ps[:],
                        w_ap[:, kp : kp + 2, :],
                        x8_ap(kp, jc * NQ, jc * NQ + nq),
                        start=(kp == 0),
                        stop=(kp == 2),
                        perf_mode=DR,
                    )
                if cast_eng is nc.scalar:
                    nc.scalar.activation(
                        out=dst[:, jc * NQ : (jc + 1) * NQ], in_=ps[:], func=AF.Copy
                    )
                else:
                    cast_eng.tensor_copy(out=dst[:, jc * NQ : (jc + 1) * NQ], in_=ps[:])

            def emit_gates(ct):
                ps = ps_proj.tile([P, NQ], F32, tag="proj", name="proj")
                for kp in (0, 2):
                    nc.tensor.matmul(
                        ps[:],
                        wg8[:, kp : kp + 2, ct * P : (ct + 1) * P],
                        x8_ap(kp, 0, NQ),
                        start=(kp == 0),
                        stop=(kp == 2),
                        perf_mode=DR,
                    )
                return ps  # scalar exp emitted separately (emit_eg)

            def emit_eg(ct, ps):
                # eg = exp(-(g + bg)); gates are fp8 with weights scaled by
                # 64, so the exp descale is -1/64. Reuses the Exp table.
                nc.scalar.activation(
                    out=egT[ct][:], in_=ps[:], func=AF.Exp, scale=-1.0 / W_SCALE,
                    bias=bgn_sb[:, ct : ct + 1],
                )
                # egp1 = 1 + eg, in place (f16, 2x DVE mode). NOT gpsimd:
                # its tensor_scalar software path measures ~7.5us/tile.
                nc.vector.tensor_scalar_add(egT[ct][:], egT[ct][:], 1.0)

            def emit_v(jt):
                # fp8 DoubleRow: v_psum = 64 * v  (descale via ones=64)
                ps = ps_proj.tile([P, NQ], F32, tag="proj", name="proj")
                for kp in (0, 2):
                    nc.tensor.matmul(
                        ps[:],
                        x8_ap(kp, jt * P, (jt + 1) * P),
                        wv8[:, kp : kp + 2, :],
                        start=(kp == 0),
                        stop=(kp == 2),
                        perf_mode=DR,
                    )
                return ps

            def emit_v_cast(jt, ps):
                nc.vector.tensor_copy(
                    out=v_aug[jt].rearrange("p (h c) -> p h c", c=P)[:, :, 0:DH],
                    in_=ps[:].rearrange("p (h c) -> p h c", c=DH),
                )

            def emit_av_jt(av, hp, jt):
                tiles = tiles_by_hp[hp]
                for s in range(2):
                    h = 2 * hp + s
                    nc.tensor.matmul(
                        av[:, s * NQ : (s + 1) * NQ],
                        v_aug[jt][:, h * P : (h + 1) * P],
                        tiles[jt][:, s * NQ : (s + 1) * NQ],
                        start=(jt == 0),
                        stop=(jt == JT - 1),
                    )

            # gated = av[v] / (denom * (1 + e^-g)) ; one fast reciprocal.
            # Split into three pieces so hp2's chain can be spread across
            # the last phase-2 slots (5 back-to-back DVE ops would delay
            # the hp3 eb-mults behind them in the DVE FIFO, stalling the
            # final exps)
            def emit_gating_a(av, hp):
                cden = rec_pool.tile([P, NQ], F32, tag="cden", name="cden")
                for s in range(2):
                    lo = s * DH
                    nc.vector.tensor_tensor(
                        cden[lo : lo + DH, :],
                        av[DH:P, s * NQ : (s + 1) * NQ],
                        egT[hp][lo : lo + DH, :],
                        ALU.mult,
                    )
                return cden

            def emit_gating_b(cden):
                crec = rec_pool.tile([P, NQ], F32, tag="crec", name="crec")
                nc.vector.reciprocal_approx_fast(out=crec[:], in_=cden[:])
                return crec

            def emit_gating_c(av, hp, crec):
                for s in range(2):
                    lo = s * DH
                    nc.vector.tensor_tensor(
                        gatedT[hp][lo : lo + DH, :],
                        av[0:DH, s * NQ : (s + 1) * NQ],
                        crec[lo : lo + DH, :],
                        ALU.mult,
                    )

            def emit_gating(av, hp):
                emit_gating_c(av, hp, emit_gating_b(emit_gating_a(av, hp)))

            # av-step iterator state: one AV accumulation step = both heads
            # of one kv tile; gating is emitted right after a pair completes
            av_state = {"a": 0, "tiles": {}}

            def emit_av_step(ps_av):
                a = av_state["a"]
                hp, jt = divmod(a, JT)
                if jt == 0:
                    av_state["tiles"][hp] = ps_av.tile(
                        [P, 2 * NQ], F32, tag="av", name="av"
                    )
                av = av_state["tiles"][hp]
                emit_av_jt(av, hp, jt)
                if jt == JT - 1:
                    emit_gating(av, hp)
                av_state["a"] = a + 1

            # PSUM is a two-sided stack: dots lives on the right and closes
            # mid-stream (its banks become the y accumulators); proj -> av
            # run sequentially on the left. Manual pool lifetimes keep both
            # sides at <= 4 banks, 8 total.
            ctx_dots = tc.tile_pool(name="ps_dots", bufs=2, space="PSUM", side="right")
            ps_dots = ctx_dots.__enter__()
            if True:
                if True:
                    # ---- phase 1: projections + dots(hp0, hp1) -----------
                    with tc.tile_pool(
                        name="ps_proj", bufs=4, space="PSUM", side="left"
                    ) as ps_proj:
                        # dummy 1x1 exp: pulls the scalar engine's
                        # ACT_TABLE_LOAD into the DMA wait at startup
                        nc.scalar.activation(
                            out=egT[0][0:1, 0:1], in_=warm_sb[0:1, 0:1],
                            func=AF.Exp,
                        )
                        warm_ps = ps_proj.tile([P, NQ], F32, tag="proj", name="proj")
                        for _ in range(N_WARM):
                            nc.tensor.matmul(
                                warm_ps[:, 0:P], warm_sb[:], warm_sb[:],
                                start=True, stop=True,
                            )
                        nc.scalar.activation(
                            out=warm_sb[0:1, 0:1], in_=warm_ps[0:1, 0:1],
                            func=AF.Copy,
                        )

                        # prime the bias ring (2 loads in flight ahead of use)
                        dispatch_eb()
                        dispatch_eb()

                        # qT0 cast on scalar (no exps exist yet); k casts on
                        # vector. All hp0 dots only need ct0; the ps_dots
                        # WAR ring paces the PE to the exp stream and
                        # fillers slot into the slack.
                        emit_kq_proj(wq_ap[0], qT[0], 0, NQ, nc.scalar)
                        emit_kq_proj(wk_ap[0], kT[0], 0, NQ, nc.vector)
                        emit_kq_proj(wk_ap[0], kT[0], 1, NQ, nc.vector)
                        emit_dots_tile(0, 0)
                        emit_dots_tile(0, 1)
                        emit_dots_tile(0, 2)
                        for ct in range(1, DT):
                            emit_kq_proj(wk_ap[ct], kT[ct], 0, NQ, nc.vector)
                            emit_kq_proj(wk_ap[ct], kT[ct], 1, NQ, nc.vector)
                            emit_kq_proj(wq_ap[ct], qT[ct], 0, NQ, nc.vector)
                            emit_dots_tile(0, 2 + ct)
                        # V-block first (its casts keep the DVE FIFO light),
                        # gates/eg at the end: the eg exps read the last
                        # proj-pool tiles on the SCALAR engine, so the
                        # phase-2 ps_av WAR never waits on a clogged DVE
                        emit_v_cast(0, emit_v(0))
                        emit_dots_tile(0, 6)
                        emit_v_cast(1, emit_v(1))
                        emit_dots_tile(0, 7)
                        emit_v_cast(2, emit_v(2))
                        emit_dots_tile(1, 0)
                        emit_v_cast(3, emit_v(3))
                        emit_dots_tile(1, 1)
                        emit_v_cast(4, emit_v(4))
                        emit_dots_tile(1, 2)
                        emit_v_cast(5, emit_v(5))
                        emit_hi_dmas()
                        emit_dots_tile(1, 3)
                        emit_v_cast(6, emit_v(6))
                        emit_dots_tile(1, 4)
                        emit_v_cast(7, emit_v(7))
                        emit_dots_tile(1, 5)
                        emit_eg(0, emit_gates(0))
                        emit_dots_tile(1, 6)
                        emit_eg(1, emit_gates(1))
                        emit_dots_tile(1, 7)
                        emit_eg(2, emit_gates(2))
                        emit_eg(3, emit_gates(3))

                    # ---- phase 2: zip remaining dots with AV steps -------
                    # [dots, av, av]: the AV stream (32 steps, starting 16
                    # behind) catches up to the exps by the end, so the tail
                    # after the last exp is just the final gating chain.
                    ctx_av = tc.tile_pool(
                        name="ps_av", bufs=2, space="PSUM", side="left"
                    )
                    ps_av = ctx_av.__enter__()
                    # dots FIRST in each slot: the exp stream must never
                    # queue behind the av catch-up in the PE FIFO
                    # dots tiles go out in PAIRS (the 2-deep psum ring allows
                    # two outstanding): dots(n+1) is no longer queued behind
                    # av steps in the PE FIFO, so the exp stream isn't
                    # stretched by the +0.86us av batch per ring handoff
                    d_emitted = 16
                    for hp in (2, 3):
                        for jt in range(JT):
                            emit_dots_tile(hp, jt)
                            d_emitted += 1
                            for _ in range(2):
                                if av_state["a"] <= d_emitted - 2:
                                    emit_av_step(ps_av)

            # dots pool closes here; its right-side banks become the y tiles
            ctx_dots.__exit__(None, None, None)
            ctx_y = tc.tile_pool(name="ps_y", bufs=1, space="PSUM", side="right")
            ps_y = ctx_y.__enter__()

            # ---- phase 3: output projection tail -------------------------
            # ct0..2 interleave with the remaining AV steps: the PE fills
            # its exp-wait slack with output-projection work
            ys = [ps_y.tile([P, DIM], F32, tag=f"y{it}", name="y") for it in range(4)]
            for ct in range(DT - 1):
                for it in range(NQ // P):
                    nc.tensor.matmul(
                        ys[it][:],
                        gatedT[ct][:, it * P : (it + 1) * P],
                        woT[:, ct, :],
                        start=(ct == 0),
                        stop=False,
                    )
                for _ in range(2):
                    if av_state["a"] < HP * JT:
                        emit_av_step(ps_av)
            # ct3 split into two 64-row halves (tile_position pairs) so the
            # it-tiles can start as soon as gated3's s0 half lands; y casts
            # pipeline with the s1 matmuls
            ysb = yout.tile([P, NQ // P, DIM], F16, tag="ysb", name="ysb")
            ydst = y_d.rearrange("(f p) m -> p f m", p=P)
            for s in range(2):
                lo = s * DH
                for it in range(NQ // P):
                    nc.tensor.matmul(
                        ys[it][:],
                        gatedT[DT - 1][lo : lo + DH, it * P : (it + 1) * P],
                        woT[lo : lo + DH, DT - 1, :],
                        start=False,
                        stop=(s == 1),
                        tile_position=(lo, 0),
                    )
                    if s == 1:
                        if it % 2 == 0:
                            nc.scalar.activation(
                                out=ysb[:, it, :], in_=ys[it][:], func=AF.Copy
                            )
                            nc.scalar.dma_start(
                                ydst[:, it : it + 1, :], ysb[:, it : it + 1, :]
                            )
                        else:
                            nc.vector.tensor_copy(out=ysb[:, it, :], in_=ys[it][:])
                            nc.sync.dma_start(
                                ydst[:, it : it + 1, :], ysb[:, it : it + 1, :]
                            )
            ctx_y.__exit__(None, None, None)
            ctx_av.__exit__(None, None, None)

    nc.compile()
    return nc


_CACHE = {}


def get_nc():
    if "nc" not in _CACHE:
        _CACHE["nc"] = build_nc()
    return _CACHE["nc"]


def make_in_maps(x, attn_bias, wq, wkv, wo, wg, bg):
    """Host-side sharding: per-core input dicts (weights shared by reference)."""
    x = np.asarray(x, np.float32)
    attn_bias = np.asarray(attn_bias, np.float32)
    wqT = np.asarray(wq, np.float32).T
    wkvT = np.asarray(wkv, np.float32).T
    wq8 = (wqT * W_SCALE).astype(FP8_NP)
    wk8 = (wkvT[:, :DIM] * W_SCALE).astype(FP8_NP)
    wv8 = (wkvT[:, DIM:] * W_SCALE).astype(FP8_NP)
    wg8 = (np.asarray(wg, np.float32).T * W_SCALE).astype(FP8_NP)
    woT = np.ascontiguousarray(np.asarray(wo, np.float32).T, np.float16)
    bgn = -np.asarray(bg, np.float32)
    f8b = np.ascontiguousarray(np.concatenate([wg8, wv8], axis=1))

    ab = np.exp(attn_bias[0])  # [H, N(i), N(j)]
    # bT[r0][h, j, i] = exp(bias)[h, i, j] with j permuted "query half first"
    bT = {}
    for r0 in (0, NQ):
        perm = np.r_[r0 : r0 + NQ, (NQ - r0) : (NQ - r0) + NQ]
        t = ab[:, r0 : r0 + NQ, :].transpose(0, 2, 1)[:, perm, :]
        t = t.reshape(4, 2, 4, 2, 128, NQ).transpose(0, 2, 4, 3, 1, 5)
        # t: [hp, pr, P, jt%2, s, NQ]
        t = np.ascontiguousarray(t, dtype=np.float16).reshape(4, 4, P, 2, 2 * NQ)
        bT[r0] = t

    in_maps = []
    for c in range(N_CORES):
        b, r0 = c // 2, (c % 2) * NQ
        perm = np.r_[r0 : r0 + NQ, (NQ - r0) : (NQ - r0) + NQ]
        xperm = x[b][perm].T
        x8 = xperm.astype(FP8_NP)
        f8a = np.concatenate(
            [
                x8[:, 0:NQ], wk8[:, 0:P], wq8[:, 0:P],  # head: lands first
                x8[:, NQ:],                              # x kv-half
                wk8[:, P:], wq8[:, P:],                  # ct1-3 weights
            ],
            axis=1,
        )
        in_maps.append(
            {
                "f8a": np.ascontiguousarray(f8a),
                "f8b": f8b,
                "woT": woT,
                "bgn": bgn,
                "bT": bT[r0],
            }
        )
    return in_maps


def kernel(x, mask, attn_bias, wq, wkv, wo, bo, wg, bg, **_):
    # mask is all-ones per the problem spec; ignored.
    nc = get_nc()
    in_maps = make_in_maps(x, attn_bias, wq, wkv, wo, wg, bg)
    res = run_bass_kernel_spmd(nc, in_maps, list(range(N_CORES))).results
    y = np.empty((B, N, DIM), np.float32)
    for c in range(N_CORES):
        b, r0 = c // 2, (c % 2) * NQ
        y[b, r0 : r0 + NQ] = res[c]["y"].astype(np.float32)
    y += np.asarray(bo, np.float32)
    return y
